# revision 37
# baseline (speedup 1.0000x reference)
"""LTC/NCP RNN (BasicRNNClassifier) Trainium2 Bass kernel.

Strategy: pure data parallel over batch (256 -> 8 cores x 32).
Per core, the sequential T=4096 recurrence runs with:
  - synapse pairs (i,j) laid out on 121 SBUF partitions
  - PE matmuls for partition-broadcast of v (sigma folded into the
    broadcast matrix) and for the masked/weighted reductions over i
    (w*mask*(erev|1) folded into a constant [121,22] matrix)
  - ACT sigmoid with per-partition bias (-mu*sigma)
  - DVE for the semi-implicit Euler update (mul/add/reciprocal/mul)
  - sensory synapses are v-independent: batched per 16-step chunk

Host->device transfer over the axon tunnel (~40MB/s, ~80ms RTT) is the
wall-clock bottleneck, so:
  - inputs ship as int18 fixed point (int16 high + 2-bit low packed
    4/byte = 2.25B/elem). The recurrence sits on a chaotic error floor:
    int18/int20/int24 all land at ~1e-3 output error vs the 2e-2 gate,
    while int16/int17 blow up to ~1e-2 — int18 is the cheapest safe
    format. The input affine map and the quantization scale are folded
    into the sensory constants.
  - the output only depends on t < seq_len[b], so the batch is sorted by
    seq_len, grouped 32-per-core, and the time axis is cut into 1024-step
    segments; a segment is only encoded+transferred for cores whose max
    seq_len reaches it (~62% of bytes on uniform lengths). v and the
    gathered output chain between segment calls as device-resident
    arrays (no per-segment host round trips); inactive cores re-run on
    their segment-0 data (finite garbage, masked out of the gather).
  - the seq_lengths gather runs on device and only [1,BC] per core is
    fetched once at the end (each blocking fetch costs a full RTT);
    the PJRT executable is built once and cached (run_bass_kernel_spmd
    re-jits per call).
"""

import numpy as np

U = 11
S = 15
F = 16
MOTOR = 1
UNFOLDS = 6
EPS = 1e-8
B, T = 256, 4096
NCORES = 8
BC = B // NCORES          # 32 batch per core
CHUNK = 16                # timesteps per inner loop iteration
SEG = 1024                # timesteps per device call (transfer skip unit)
W = CHUNK * BC            # 512 columns per chunk
QMAX = float(2**17 - 1)   # int18 quantization (int16 hi + 2-bit lo, 4/byte)


# packed constant block: name -> (rows, col_offset, cols, flat_offset)
_sizes = [("sigB", U, U * U), ("gw", U * U, 43), ("i43", 43, 43),
          ("sigBsA", S, 88), ("sigBsB", S, 77), ("gwsA", 88, 43),
          ("gwsB", 77, 43), ("aug", 1, 43), ("cm6", 1, U),
          ("negmusig", U * U, 1), ("nmsA", 88, 1), ("nmsB", 77, 1)]
CB_LAYOUT = {}
_off = 0
_foff = 0
for _n, _r, _c in _sizes:
    CB_LAYOUT[_n] = (_r, _off, _c, _foff)
    _off += _c
    _foff += _r * _c
CB_COLS = _off
CB_FLAT = _foff

_cache = {}


def _build(t_steps, chunk):
    import concourse.bass as bass
    import concourse.tile as tile
    import concourse.mybir as mybir
    from concourse import bacc
    from contextlib import ExitStack

    import concourse.tile_sem_assignment as _tsa
    _tsa.NUM_HWDGE_SEMS = 1   # keep the loop back-edge barrier under the
                              # per-instruction sync-wait limit

    f32 = mybir.dt.float32
    i16 = mybir.dt.int16
    u8 = mybir.dt.uint8
    nch = t_steps // chunk
    w = chunk * BC

    nc = bacc.Bacc("TRN2", target_bir_lowering=False, debug=False)

    hi_d = nc.dram_tensor("hi", [F, t_steps * BC], i16, kind="ExternalInput").ap()
    lp_d = nc.dram_tensor("lp", [F, t_steps * BC // 4], u8,
                          kind="ExternalInput").ap()
    cbf_d = nc.dram_tensor("cbf", [1, CB_FLAT], f32, kind="ExternalInput").ap()
    tsel_d = nc.dram_tensor("tsel", [1, BC], f32, kind="ExternalInput").ap()
    tg_d = nc.dram_tensor("tg", [1, t_steps], f32, kind="ExternalInput").ap()
    vin_d = nc.dram_tensor("vin", [U, BC], f32, kind="ExternalInput").ap()
    yin_d = nc.dram_tensor("yin", [1, BC], f32, kind="ExternalInput").ap()
    ysel_d = nc.dram_tensor("ysel", [1, BC], f32, kind="ExternalOutput").ap()
    vout_d = nc.dram_tensor("vout", [U, BC], f32, kind="ExternalOutput").ap()

    with ExitStack() as ctx:
        tc = ctx.enter_context(tile.TileContext(nc))

        cpool = ctx.enter_context(tc.tile_pool(name="consts", bufs=1))
        vpool = ctx.enter_context(tc.tile_pool(name="vstate", bufs=1))
        xpool = ctx.enter_context(tc.tile_pool(name="xin", bufs=2))
        spool = ctx.enter_context(tc.tile_pool(name="sens", bufs=2))
        apool = ctx.enter_context(tc.tile_pool(name="acts", bufs=3))
        tpool = ctx.enter_context(tc.tile_pool(name="tmps", bufs=3))
        pp_s = ctx.enter_context(tc.tile_pool(name="ps_sens", bufs=1, space="PSUM"))
        pp_u = ctx.enter_context(tc.tile_pool(name="ps_unf", bufs=2, space="PSUM"))

        cb = cpool.tile([128, CB_COLS], f32, tag="cb")
        for _name, (r, o, n, fo) in CB_LAYOUT.items():
            nc.sync.dma_start(cb[0:r, o:o + n], cbf_d[0:1, fo:fo + r * n])
        c = {k: cb[0:r, o:o + n] for k, (r, o, n, _fo) in CB_LAYOUT.items()}

        tsel_sb = cpool.tile([1, BC], f32, tag="tsel")
        nc.sync.dma_start(tsel_sb[:], tsel_d[:])

        # 2-bit-unpack constants (per-partition uint8 scalars so the
        # bitwise ALU ops stay in the integer domain)
        cnib = cpool.tile([33, 4], u8, tag="cnib")
        nc.vector.memset(cnib[:, 0:1], 3)
        nc.vector.memset(cnib[:, 1:2], 2)
        nc.vector.memset(cnib[:, 2:3], 4)
        nc.vector.memset(cnib[:, 3:4], 6)

        ones = cpool.tile([1, w], f32, tag="ones")
        nc.vector.memset(ones[:], 1.0)
        va = vpool.tile([U, BC], f32, tag="va")
        vb = vpool.tile([U, BC], f32, tag="vb")
        nc.sync.dma_start(va[:], vin_d[:])
        ysel = vpool.tile([1, BC], f32, tag="ysel")
        nc.sync.dma_start(ysel[:], yin_d[:])

        sig = mybir.ActivationFunctionType.Sigmoid
        cp = mybir.ActivationFunctionType.Copy
        mult = mybir.AluOpType.mult
        add = mybir.AluOpType.add
        iseq = mybir.AluOpType.is_equal

        with tc.For_i(0, nch, 1,
                      hint_engines=(mybir.EngineType.PE, mybir.EngineType.DVE)) as ci:
            # engine reads must start at partition 0/32/64/96, so the
            # elapsed-time row lands on partition 32 (features on 0..14)
            hi_sb = xpool.tile([33, w], i16, tag="hi")
            nc.sync.dma_start(hi_sb[0:S, :], hi_d[0:S, bass.ts(ci, w)])
            nc.sync.dma_start(hi_sb[32:33, :], hi_d[S:S + 1, bass.ts(ci, w)])
            qw = w // 4
            lp_sb = xpool.tile([33, qw], u8, tag="lp")
            nc.sync.dma_start(lp_sb[0:S, :], lp_d[0:S, bass.ts(ci, qw)])
            nc.sync.dma_start(lp_sb[32:33, :], lp_d[S:S + 1, bass.ts(ci, qw)])
            tg_sb = xpool.tile([1, chunk], f32, tag="tg")
            nc.sync.dma_start(tg_sb[:], tg_d[:, bass.ts(ci, chunk)])

            # unpack 2-bit fields: byte j packs cols (j, j+128, j+256,
            # j+384) of this chunk
            band = mybir.AluOpType.bitwise_and
            bshr = mybir.AluOpType.logical_shift_right
            lo_sb = xpool.tile([33, w], u8, tag="lo")
            for base in (0, 32):
                pr = slice(base, base + (S if base == 0 else 1))
                nc.vector.tensor_scalar(lo_sb[pr, 0:qw], lp_sb[pr, :],
                                        cnib[pr, 0:1], None, band)
                nc.vector.tensor_scalar(lo_sb[pr, qw:2 * qw], lp_sb[pr, :],
                                        cnib[pr, 1:2], cnib[pr, 0:1],
                                        bshr, op1=band)
                nc.vector.tensor_scalar(lo_sb[pr, 2 * qw:3 * qw], lp_sb[pr, :],
                                        cnib[pr, 2:3], cnib[pr, 0:1],
                                        bshr, op1=band)
                nc.vector.tensor_scalar(lo_sb[pr, 3 * qw:w], lp_sb[pr, :],
                                        cnib[pr, 3:4], None, bshr)

            # decode int18: xq = hi*4 + lo (exact in f32; scale folded
            # into sigBs/cm6 constants)
            xq = xpool.tile([33, w], f32, tag="xq")
            nc.scalar.activation(xq[0:S, :], hi_sb[0:S, :], cp, scale=4.0)
            nc.vector.scalar_tensor_tensor(xq[0:S, :], lo_sb[0:S, :], 1.0,
                                           xq[0:S, :], op0=mult, op1=add)
            nc.scalar.activation(xq[32:33, :], hi_sb[32:33, :], cp, scale=4.0)
            nc.vector.scalar_tensor_tensor(xq[32:33, :], lo_sb[32:33, :], 1.0,
                                           xq[32:33, :], op0=mult, op1=add)

            # sensory synapses, batched over the whole chunk
            pA = pp_s.tile([88, w], f32, tag="pA")
            nc.tensor.matmul(pA[:], c["sigBsA"][:], xq[0:S, :], start=True, stop=True)
            aA = spool.tile([88, w], f32, tag="aA")
            nc.scalar.activation(aA[:], pA[:], sig, bias=c["nmsA"][:])
            pB = pp_s.tile([77, w], f32, tag="pB")
            nc.tensor.matmul(pB[:], c["sigBsB"][:], xq[0:S, :], start=True, stop=True)
            aB = spool.tile([77, w], f32, tag="aB")
            nc.scalar.activation(aB[:], pB[:], sig, bias=c["nmsB"][:])

            p_nd1 = pp_s.tile([43, w], f32, tag="pnd1")
            nc.tensor.matmul(p_nd1[:], c["gwsA"][:], aA[:], start=True, stop=False)
            nc.tensor.matmul(p_nd1[:], c["gwsB"][:], aB[:], start=False, stop=False)
            nc.tensor.matmul(p_nd1[:], c["aug"][:], ones[:], start=False, stop=True)

            # cm_t = (UNFOLDS*cm/scale) * (1/xq_elapsed)
            rec = tpool.tile([1, w], f32, tag="rec")
            nc.vector.reciprocal(rec[:], xq[32:33, :])
            p_cm = pp_s.tile([U, w], f32, tag="pcm")
            nc.tensor.matmul(p_cm[:], c["cm6"][:], rec[:], start=True, stop=True)
            cmt = spool.tile([U, w], f32, tag="cmt")
            nc.vector.tensor_copy(cmt[:], p_cm[:])

            nd1 = spool.tile([43, w], f32, tag="nd1")
            nc.vector.tensor_copy(nd1[:], p_nd1[:])
            nc.vector.tensor_add(nd1[32:43, :], p_nd1[32:43, :], cmt[:])

            vcur = va
            for s in range(chunk):
                col = slice(s * BC, (s + 1) * BC)
                for k in range(UNFOLDS):
                    p_nd = pp_u.tile([43, BC], f32, tag="pnd")
                    nc.tensor.matmul(p_nd[:], c["i43"][:], nd1[:, col],
                                     start=True, stop=False)
                    p_vr = pp_u.tile([U * U, BC], f32, tag="pvr")
                    nc.tensor.matmul(p_vr[:], c["sigB"][:], vcur[:],
                                     start=True, stop=True)
                    act = apool.tile([U * U, BC], f32, tag="act")
                    nc.scalar.activation(act[:], p_vr[:], sig, bias=c["negmusig"][:])
                    nc.tensor.matmul(p_nd[:], c["gw"][:], act[:],
                                     start=False, stop=True)

                    t1 = tpool.tile([U, BC], f32, tag="t1")
                    nc.vector.tensor_mul(t1[:], cmt[:, col], vcur[:])
                    numer = tpool.tile([U, BC], f32, tag="numer")
                    nc.vector.tensor_add(numer[:], t1[:], p_nd[0:U, :])
                    rcp = tpool.tile([U, BC], f32, tag="rcp")
                    nc.vector.reciprocal(rcp[:], p_nd[32:43, :])
                    vnext = vb if k % 2 == 0 else va
                    nc.vector.tensor_mul(vnext[:], numer[:], rcp[:])
                    vcur = vnext

                # seq_lengths gather: ysel += v_motor * (t == tsel)
                mask = tpool.tile([1, BC], f32, tag="mask")
                nc.vector.tensor_scalar(mask[:], tsel_sb[:], tg_sb[0:1, s:s + 1],
                                        None, iseq)
                sel = tpool.tile([1, BC], f32, tag="sel")
                nc.vector.tensor_mul(sel[:], mask[:], vcur[0:1, :])
                nc.vector.tensor_add(ysel[:], ysel[:], sel[:])

        nc.sync.dma_start(ysel_d[:], ysel[:])
        nc.sync.dma_start(vout_d[:], va[:])

    nc.compile()
    return nc


def _prep_consts(p, scale):
    """Constant matrices from the parameter dict; the input affine map
    (input_w/input_b) and the int18 scale are folded into the sensory
    broadcast matrix and bias."""
    iU = np.arange(U)
    sigB = np.zeros((U, U * U), np.float32)
    sigB[iU[:, None], iU[:, None] * U + iU[None, :]] = p["sigma"]
    negmusig = (-(p["mu"] * p["sigma"]).reshape(U * U, 1)).astype(np.float32)
    wm = p["w"] * p["sparsity_mask"]
    gw = np.zeros((U * U, 43), np.float32)
    flat = np.arange(U * U)
    jj = flat % U
    gw[flat, jj] = (wm * p["erev"]).reshape(-1)
    gw[flat, 32 + jj] = wm.reshape(-1)
    i43 = np.eye(43, dtype=np.float32)

    iS = np.arange(S)
    # arg = xq*(scale*input_w*sigma) + (input_b - mu)*sigma
    sig_eff = (p["sensory_sigma"] * p["input_w"][:, None] * scale).astype(np.float32)
    sigBs = np.zeros((S, S * U), np.float32)
    sigBs[iS[:, None], iS[:, None] * U + iU[None, :]] = sig_eff
    nms = ((p["input_b"][:, None] - p["sensory_mu"]) * p["sensory_sigma"]
           ).reshape(S * U, 1).astype(np.float32)
    swm = p["sensory_w"] * p["sensory_sparsity_mask"]
    gws = np.zeros((S * U, 43), np.float32)
    sflat = np.arange(S * U)
    uu = sflat % U
    gws[sflat, uu] = (swm * p["sensory_erev"]).reshape(-1)
    gws[sflat, 32 + uu] = swm.reshape(-1)

    aug = np.zeros((1, 43), np.float32)
    aug[0, :U] = p["gleak"] * p["vleak"]
    aug[0, 32:43] = p["gleak"] + EPS
    cm6 = (UNFOLDS * p["cm"] / scale).reshape(1, U).astype(np.float32)

    mats = {
        "sigB": sigB, "negmusig": negmusig, "gw": gw, "i43": i43,
        "sigBsA": sigBs[:, :88], "sigBsB": sigBs[:, 88:],
        "nmsA": nms[:88], "nmsB": nms[88:],
        "gwsA": gws[:88], "gwsB": gws[88:],
        "aug": aug, "cm6": cm6,
    }
    cbf = np.zeros(CB_FLAT, np.float32)
    for k, (r, _o, n, fo) in CB_LAYOUT.items():
        cbf[fo:fo + r * n] = np.ascontiguousarray(mats[k], np.float32).reshape(-1)
    return cbf


def _get_runner(nc):
    """Build (once) a cached jitted shard_map executor for nc — the
    equivalent of bass_utils.run_bass_kernel_spmd's axon path, minus the
    per-call retrace/re-jit."""
    import jax
    from jax.sharding import Mesh, PartitionSpec, NamedSharding
    from jax.experimental.shard_map import shard_map
    import concourse.mybir as mybir
    from concourse import bass2jax

    bass2jax.install_neuronx_cc_hook()

    partition_name = nc.partition_id_tensor.name if nc.partition_id_tensor else None
    in_names, out_names, out_avals, zero_outs = [], [], [], []
    for alloc in nc.m.functions[0].allocations:
        if not isinstance(alloc, mybir.MemoryLocationSet):
            continue
        name = alloc.memorylocations[0].name
        if alloc.kind == "ExternalInput":
            if name != partition_name:
                in_names.append(name)
        elif alloc.kind == "ExternalOutput":
            shape = tuple(alloc.tensor_shape)
            dtype = mybir.dt.np(alloc.dtype)
            out_names.append(name)
            out_avals.append(jax.core.ShapedArray(shape, dtype))
            zero_outs.append(np.zeros(shape, dtype))
    n_params = len(in_names)
    n_outs = len(out_avals)
    in_names_all = list(in_names) + out_names
    if partition_name is not None:
        in_names_all.append(partition_name)
    donate = tuple(range(n_params, n_params + n_outs))

    def _body(*args):
        operands = list(args)
        if partition_name is not None:
            operands.append(bass2jax.partition_id_tensor())
        outs = bass2jax._bass_exec_p.bind(
            *operands,
            out_avals=tuple(out_avals),
            in_names=tuple(in_names_all),
            out_names=tuple(out_names),
            lowering_input_output_aliases=(),
            sim_require_finite=True,
            sim_require_nnan=True,
            nc=nc,
        )
        return tuple(outs)

    devices = jax.devices()[:NCORES]
    assert len(devices) == NCORES
    mesh = Mesh(np.asarray(devices), ("core",))
    in_specs = (PartitionSpec("core"),) * (n_params + n_outs)
    out_specs = (PartitionSpec("core"),) * n_outs
    sharded = jax.jit(
        shard_map(_body, mesh=mesh, in_specs=in_specs, out_specs=out_specs,
                  check_rep=False),
        donate_argnums=donate,
        keep_unused=True,
    )
    sh = NamedSharding(mesh, PartitionSpec("core"))
    return sharded, in_names, out_names, zero_outs, devices, sh


def _schedule(total):
    """Split `total` (a positive multiple of 512) into call sizes from
    {1024, 512}, preferring 1024 pieces but ending in 512s so the tail
    call (and per-core granularity overshoot) is small."""
    sizes = []
    rest = total
    while rest > 1536:
        sizes.append(1024)
        rest -= 1024
    while rest > 0:
        sizes.append(512)
        rest -= 512
    return sizes


def _get_built(size):
    key = (size, CHUNK)
    if key not in _cache:
        nc = _build(size, CHUNK)
        _cache[key] = (nc, _get_runner(nc))
    return _cache[key]


_dummies = {}


def _dummy_piece(jax, cid, dev, size):
    """Device-resident filler for cores that never ship a piece of this
    size: features zero, elapsed-row positive (avoids inf/nan in the
    recurrence; the gather mask is always 0 for these cores)."""
    k = (cid, size)
    if k not in _dummies:
        hi = np.zeros((F, size * BC), np.int16)
        hi[S, :] = 1
        lp = np.zeros((F, size * BC // 4), np.uint8)
        _dummies[k] = (jax.device_put(hi, dev), jax.device_put(lp, dev))
    return _dummies[k]


def _encode_piece(x, bidx, start, size, inv):
    """int18-encode one (core, segment) slice: int16 hi + 2-bit lo packed
    4-per-byte (byte j of a 512-col chunk holds cols j, j+128, j+256,
    j+384)."""
    xs = x[bidx, start:start + size, :]                  # [BC, size, F]
    q = np.rint(xs * inv).astype(np.int32)
    hi = np.ascontiguousarray(
        (q >> 2).astype(np.int16).transpose(2, 1, 0)
    ).reshape(F, size * BC)
    lo2 = np.ascontiguousarray(
        (q & 0x3).astype(np.uint8).transpose(2, 1, 0)
    ).reshape(F, size * BC // W, 4, W // 4)
    lo = np.ascontiguousarray(
        lo2[:, :, 0, :] | (lo2[:, :, 1, :] << 2)
        | (lo2[:, :, 2, :] << 4) | (lo2[:, :, 3, :] << 6)
    ).reshape(F, size * BC // 4)
    return hi, lo


def kernel(**inputs):
    import jax

    p = {}
    for k, v in inputs.items():
        a = np.asarray(v)
        p[k] = a if a.dtype in (np.int64, np.int32) else np.asarray(a, np.float32)
    seq_lengths = np.asarray(inputs["seq_lengths"]).astype(np.int64)
    x = np.asarray(inputs["inputs"], np.float32)            # [B, T, F]

    # sort batch by seq_len so short sequences share a core and late
    # segments need no data at all for short cores
    perm = np.argsort(seq_lengths, kind="stable")
    seq_s = seq_lengths[perm]
    tmax_core = seq_s.reshape(NCORES, BC).max(axis=1)
    total = int(-(-int(tmax_core.max()) // 512) * 512)
    sizes = _schedule(total)

    nc, (sharded, in_names, out_names, zero_outs, devs, shd) = _get_built(sizes[0])

    # int18 encode: x ~= (hi*4 + lo_2bit) * scale
    # (threaded abs-max: numpy reductions release the GIL)
    import concurrent.futures as _cf
    xf = x.reshape(-1)
    nth = 8
    stride = (xf.shape[0] + nth - 1) // nth
    with _cf.ThreadPoolExecutor(nth) as _ex:
        mins = list(_ex.map(lambda i: float(xf[i * stride:(i + 1) * stride].min()),
                            range(nth)))
        maxs = list(_ex.map(lambda i: float(xf[i * stride:(i + 1) * stride].max()),
                            range(nth)))
    scale = max(max(maxs), -min(mins), 1e-30) / QMAX
    inv = 1.0 / scale

    cbf = _prep_consts(p, scale)
    cbf_dev = jax.device_put(np.tile(cbf.reshape(1, CB_FLAT), (NCORES, 1)), shd)
    tsel_dev = jax.device_put(
        (seq_s - 1).reshape(NCORES, BC).astype(np.float32), shd)
    v_dev = jax.device_put(np.zeros((NCORES * U, BC), np.float32), shd)
    y_dev = jax.device_put(np.zeros((NCORES, BC), np.float32), shd)

    last_piece = {}   # (cid, size) -> (hi_dev, lo_dev): reusable filler
    start = 0
    for size in sizes:
        nc, (sharded, in_names, out_names, zero_outs, devs, shd) = \
            _get_built(size)
        iy = out_names.index("ysel")
        iv = out_names.index("vout")
        extra = {}
        if nc.dbg_addr is not None:
            extra[nc.dbg_addr.name] = jax.device_put(
                np.tile(np.zeros((1, 2), np.uint32), (NCORES, 1)), shd)

        hi_shards, lo_shards = [], []
        for cid in range(NCORES):
            if start < tmax_core[cid]:
                bidx = perm[cid * BC:(cid + 1) * BC]
                hi, lo = _encode_piece(x, bidx, start, size, inv)
                hi_dev = jax.device_put(hi, devs[cid])
                lo_dev = jax.device_put(lo, devs[cid])
                last_piece[(cid, size)] = (hi_dev, lo_dev)
            elif (cid, size) in last_piece:
                # inactive core: rerun on stale data (finite garbage; its
                # gather already completed so ysel stays untouched)
                hi_dev, lo_dev = last_piece[(cid, size)]
            else:
                hi_dev, lo_dev = _dummy_piece(jax, cid, devs[cid], size)
            hi_shards.append(hi_dev)
            lo_shards.append(lo_dev)

        hi_g = jax.make_array_from_single_device_arrays(
            (NCORES * F, size * BC), shd, hi_shards)
        lo_g = jax.make_array_from_single_device_arrays(
            (NCORES * F, size * BC // 4), shd, lo_shards)
        tg = np.tile((start + np.arange(size, dtype=np.float32)
                      ).reshape(1, size), (NCORES, 1))

        feeds = {"hi": hi_g, "lp": lo_g, "cbf": cbf_dev, "tsel": tsel_dev,
                 "tg": tg, "vin": v_dev, "yin": y_dev}
        feeds.update(extra)
        args = [feeds[n] for n in in_names]
        zeros_g = [np.zeros((NCORES * z.shape[0],) + z.shape[1:], z.dtype)
                   for z in zero_outs]
        outs = sharded(*args, *zeros_g)
        v_dev = outs[iv]
        y_dev = outs[iy]
        start += size

    try:
        y_dev.copy_to_host_async()
    except Exception:
        pass
    ysel = np.asarray(y_dev).reshape(NCORES, BC)

    seq_vals = ysel.reshape(B) * p["output_w"][0] + p["output_b"][0]
    out_sorted = seq_vals * p["dense_w"][0, 0] + p["dense_b"][0]
    out = np.empty(B, np.float32)
    out[perm] = out_sorted
    return out.reshape(B, 1, 1).astype(np.float32)


# revision 41
# speedup vs baseline: 1.0351x; 1.0351x over previous
"""LTC/NCP RNN (BasicRNNClassifier) Trainium2 Bass kernel.

Strategy: pure data parallel over batch (256 -> 8 cores x 32).
Per core, the sequential T=4096 recurrence runs with:
  - synapse pairs (i,j) laid out on 121 SBUF partitions
  - PE matmuls for partition-broadcast of v (sigma folded into the
    broadcast matrix) and for the masked/weighted reductions over i
    (w*mask*(erev|1) folded into a constant [121,22] matrix)
  - ACT sigmoid with per-partition bias (-mu*sigma)
  - DVE for the semi-implicit Euler update (mul/add/reciprocal/mul)
  - sensory synapses are v-independent: batched per 16-step chunk

Host->device transfer over the axon tunnel (~40MB/s, ~80ms RTT) is the
wall-clock bottleneck, so:
  - inputs ship as int18 fixed point (int16 high + 2-bit low packed
    4/byte = 2.25B/elem). The recurrence sits on a chaotic error floor:
    int18/int20/int24 all land at ~1e-3 output error vs the 2e-2 gate,
    while int16/int17 blow up to ~1e-2 — int18 is the cheapest safe
    format. The input affine map and the quantization scale are folded
    into the sensory constants.
  - the output only depends on t < seq_len[b], so the batch is sorted by
    seq_len, grouped 32-per-core, and the time axis is cut into 1024-step
    segments; a segment is only encoded+transferred for cores whose max
    seq_len reaches it (~62% of bytes on uniform lengths). v and the
    gathered output chain between segment calls as device-resident
    arrays (no per-segment host round trips); inactive cores re-run on
    their segment-0 data (finite garbage, masked out of the gather).
  - the seq_lengths gather runs on device and only [1,BC] per core is
    fetched once at the end (each blocking fetch costs a full RTT);
    the PJRT executable is built once and cached (run_bass_kernel_spmd
    re-jits per call).
"""

import numpy as np

U = 11
S = 15
F = 16
MOTOR = 1
UNFOLDS = 6
EPS = 1e-8
B, T = 256, 4096
NCORES = 8
BC = B // NCORES          # 32 batch per core
CHUNK = 16                # timesteps per inner loop iteration
SEG = 1024                # timesteps per device call (transfer skip unit)
W = CHUNK * BC            # 512 columns per chunk
QMAX = float(2**17 - 1)   # int18 quantization (int16 hi + 2-bit lo, 4/byte)


# packed constant block: name -> (rows, col_offset, cols, flat_offset)
_sizes = [("sigB", U, U * U), ("gw", U * U, 43), ("i43", 43, 43),
          ("sigBsA", S, 88), ("sigBsB", S, 77), ("gwsA", 88, 43),
          ("gwsB", 77, 43), ("aug", 1, 43), ("cm6", 1, U),
          ("negmusig", U * U, 1), ("nmsA", 88, 1), ("nmsB", 77, 1)]
CB_LAYOUT = {}
_off = 0
_foff = 0
for _n, _r, _c in _sizes:
    CB_LAYOUT[_n] = (_r, _off, _c, _foff)
    _off += _c
    _foff += _r * _c
CB_COLS = _off
CB_FLAT = _foff

_cache = {}


def _build(t_steps, chunk):
    import concourse.bass as bass
    import concourse.tile as tile
    import concourse.mybir as mybir
    from concourse import bacc
    from contextlib import ExitStack

    import concourse.tile_sem_assignment as _tsa
    _tsa.NUM_HWDGE_SEMS = 1   # keep the loop back-edge barrier under the
                              # per-instruction sync-wait limit

    f32 = mybir.dt.float32
    i16 = mybir.dt.int16
    u8 = mybir.dt.uint8
    nch = t_steps // chunk
    w = chunk * BC

    nc = bacc.Bacc("TRN2", target_bir_lowering=False, debug=False)

    hi_d = nc.dram_tensor("hi", [F, t_steps * BC], i16, kind="ExternalInput").ap()
    lp_d = nc.dram_tensor("lp", [F, t_steps * BC // 4], u8,
                          kind="ExternalInput").ap()
    cbf_d = nc.dram_tensor("cbf", [1, CB_FLAT], f32, kind="ExternalInput").ap()
    tsel_d = nc.dram_tensor("tsel", [1, BC], f32, kind="ExternalInput").ap()
    tg_d = nc.dram_tensor("tg", [1, t_steps], f32, kind="ExternalInput").ap()
    vin_d = nc.dram_tensor("vin", [U, BC], f32, kind="ExternalInput").ap()
    yin_d = nc.dram_tensor("yin", [1, BC], f32, kind="ExternalInput").ap()
    ysel_d = nc.dram_tensor("ysel", [1, BC], f32, kind="ExternalOutput").ap()
    vout_d = nc.dram_tensor("vout", [U, BC], f32, kind="ExternalOutput").ap()

    with ExitStack() as ctx:
        tc = ctx.enter_context(tile.TileContext(nc))

        cpool = ctx.enter_context(tc.tile_pool(name="consts", bufs=1))
        vpool = ctx.enter_context(tc.tile_pool(name="vstate", bufs=1))
        xpool = ctx.enter_context(tc.tile_pool(name="xin", bufs=2))
        spool = ctx.enter_context(tc.tile_pool(name="sens", bufs=2))
        apool = ctx.enter_context(tc.tile_pool(name="acts", bufs=3))
        tpool = ctx.enter_context(tc.tile_pool(name="tmps", bufs=3))
        pp_s = ctx.enter_context(tc.tile_pool(name="ps_sens", bufs=1, space="PSUM"))
        pp_u = ctx.enter_context(tc.tile_pool(name="ps_unf", bufs=2, space="PSUM"))

        cb = cpool.tile([128, CB_COLS], f32, tag="cb")
        for _name, (r, o, n, fo) in CB_LAYOUT.items():
            nc.sync.dma_start(cb[0:r, o:o + n], cbf_d[0:1, fo:fo + r * n])
        c = {k: cb[0:r, o:o + n] for k, (r, o, n, _fo) in CB_LAYOUT.items()}

        tsel_sb = cpool.tile([1, BC], f32, tag="tsel")
        nc.sync.dma_start(tsel_sb[:], tsel_d[:])

        # 2-bit-unpack constants (per-partition uint8 scalars so the
        # bitwise ALU ops stay in the integer domain)
        cnib = cpool.tile([33, 4], u8, tag="cnib")
        nc.vector.memset(cnib[:, 0:1], 3)
        nc.vector.memset(cnib[:, 1:2], 2)
        nc.vector.memset(cnib[:, 2:3], 4)
        nc.vector.memset(cnib[:, 3:4], 6)

        ones = cpool.tile([1, w], f32, tag="ones")
        nc.vector.memset(ones[:], 1.0)
        va = vpool.tile([U, BC], f32, tag="va")
        vb = vpool.tile([U, BC], f32, tag="vb")
        nc.sync.dma_start(va[:], vin_d[:])
        ysel = vpool.tile([1, BC], f32, tag="ysel")
        nc.sync.dma_start(ysel[:], yin_d[:])

        sig = mybir.ActivationFunctionType.Sigmoid
        cp = mybir.ActivationFunctionType.Copy
        mult = mybir.AluOpType.mult
        add = mybir.AluOpType.add
        iseq = mybir.AluOpType.is_equal

        with tc.For_i(0, nch, 1,
                      hint_engines=(mybir.EngineType.PE, mybir.EngineType.DVE)) as ci:
            # engine reads must start at partition 0/32/64/96, so the
            # elapsed-time row lands on partition 32 (features on 0..14)
            hi_sb = xpool.tile([33, w], i16, tag="hi")
            nc.sync.dma_start(hi_sb[0:S, :], hi_d[0:S, bass.ts(ci, w)])
            nc.sync.dma_start(hi_sb[32:33, :], hi_d[S:S + 1, bass.ts(ci, w)])
            qw = w // 4
            lp_sb = xpool.tile([33, qw], u8, tag="lp")
            nc.sync.dma_start(lp_sb[0:S, :], lp_d[0:S, bass.ts(ci, qw)])
            nc.sync.dma_start(lp_sb[32:33, :], lp_d[S:S + 1, bass.ts(ci, qw)])
            tg_sb = xpool.tile([1, chunk], f32, tag="tg")
            nc.sync.dma_start(tg_sb[:], tg_d[:, bass.ts(ci, chunk)])

            # unpack 2-bit fields: byte j packs cols (j, j+128, j+256,
            # j+384) of this chunk
            band = mybir.AluOpType.bitwise_and
            bshr = mybir.AluOpType.logical_shift_right
            lo_sb = xpool.tile([33, w], u8, tag="lo")
            for base in (0, 32):
                pr = slice(base, base + (S if base == 0 else 1))
                nc.vector.tensor_scalar(lo_sb[pr, 0:qw], lp_sb[pr, :],
                                        cnib[pr, 0:1], None, band)
                nc.vector.tensor_scalar(lo_sb[pr, qw:2 * qw], lp_sb[pr, :],
                                        cnib[pr, 1:2], cnib[pr, 0:1],
                                        bshr, op1=band)
                nc.vector.tensor_scalar(lo_sb[pr, 2 * qw:3 * qw], lp_sb[pr, :],
                                        cnib[pr, 2:3], cnib[pr, 0:1],
                                        bshr, op1=band)
                nc.vector.tensor_scalar(lo_sb[pr, 3 * qw:w], lp_sb[pr, :],
                                        cnib[pr, 3:4], None, bshr)

            # decode int18: xq = hi*4 + lo (exact in f32; scale folded
            # into sigBs/cm6 constants)
            xq = xpool.tile([33, w], f32, tag="xq")
            nc.scalar.activation(xq[0:S, :], hi_sb[0:S, :], cp, scale=4.0)
            nc.vector.scalar_tensor_tensor(xq[0:S, :], lo_sb[0:S, :], 1.0,
                                           xq[0:S, :], op0=mult, op1=add)
            nc.scalar.activation(xq[32:33, :], hi_sb[32:33, :], cp, scale=4.0)
            nc.vector.scalar_tensor_tensor(xq[32:33, :], lo_sb[32:33, :], 1.0,
                                           xq[32:33, :], op0=mult, op1=add)

            # sensory synapses, batched over the whole chunk
            pA = pp_s.tile([88, w], f32, tag="pA")
            nc.tensor.matmul(pA[:], c["sigBsA"][:], xq[0:S, :], start=True, stop=True)
            aA = spool.tile([88, w], f32, tag="aA")
            nc.scalar.activation(aA[:], pA[:], sig, bias=c["nmsA"][:])
            pB = pp_s.tile([77, w], f32, tag="pB")
            nc.tensor.matmul(pB[:], c["sigBsB"][:], xq[0:S, :], start=True, stop=True)
            aB = spool.tile([77, w], f32, tag="aB")
            nc.scalar.activation(aB[:], pB[:], sig, bias=c["nmsB"][:])

            p_nd1 = pp_s.tile([43, w], f32, tag="pnd1")
            nc.tensor.matmul(p_nd1[:], c["gwsA"][:], aA[:], start=True, stop=False)
            nc.tensor.matmul(p_nd1[:], c["gwsB"][:], aB[:], start=False, stop=False)
            nc.tensor.matmul(p_nd1[:], c["aug"][:], ones[:], start=False, stop=True)

            # cm_t = (UNFOLDS*cm/scale) * (1/xq_elapsed)
            rec = tpool.tile([1, w], f32, tag="rec")
            nc.vector.reciprocal(rec[:], xq[32:33, :])
            p_cm = pp_s.tile([U, w], f32, tag="pcm")
            nc.tensor.matmul(p_cm[:], c["cm6"][:], rec[:], start=True, stop=True)
            cmt = spool.tile([U, w], f32, tag="cmt")
            nc.vector.tensor_copy(cmt[:], p_cm[:])

            nd1 = spool.tile([43, w], f32, tag="nd1")
            nc.vector.tensor_copy(nd1[:], p_nd1[:])
            nc.vector.tensor_add(nd1[32:43, :], p_nd1[32:43, :], cmt[:])

            vcur = va
            for s in range(chunk):
                col = slice(s * BC, (s + 1) * BC)
                for k in range(UNFOLDS):
                    p_nd = pp_u.tile([43, BC], f32, tag="pnd")
                    nc.tensor.matmul(p_nd[:], c["i43"][:], nd1[:, col],
                                     start=True, stop=False)
                    p_vr = pp_u.tile([U * U, BC], f32, tag="pvr")
                    nc.tensor.matmul(p_vr[:], c["sigB"][:], vcur[:],
                                     start=True, stop=True)
                    act = apool.tile([U * U, BC], f32, tag="act")
                    nc.scalar.activation(act[:], p_vr[:], sig, bias=c["negmusig"][:])
                    nc.tensor.matmul(p_nd[:], c["gw"][:], act[:],
                                     start=False, stop=True)

                    t1 = tpool.tile([U, BC], f32, tag="t1")
                    nc.vector.tensor_mul(t1[:], cmt[:, col], vcur[:])
                    numer = tpool.tile([U, BC], f32, tag="numer")
                    nc.vector.tensor_add(numer[:], t1[:], p_nd[0:U, :])
                    rcp = tpool.tile([U, BC], f32, tag="rcp")
                    nc.vector.reciprocal(rcp[:], p_nd[32:43, :])
                    vnext = vb if k % 2 == 0 else va
                    nc.vector.tensor_mul(vnext[:], numer[:], rcp[:])
                    vcur = vnext

                # seq_lengths gather: ysel += v_motor * (t == tsel)
                mask = tpool.tile([1, BC], f32, tag="mask")
                nc.vector.tensor_scalar(mask[:], tsel_sb[:], tg_sb[0:1, s:s + 1],
                                        None, iseq)
                sel = tpool.tile([1, BC], f32, tag="sel")
                nc.vector.tensor_mul(sel[:], mask[:], vcur[0:1, :])
                nc.vector.tensor_add(ysel[:], ysel[:], sel[:])

        nc.sync.dma_start(ysel_d[:], ysel[:])
        nc.sync.dma_start(vout_d[:], va[:])

    nc.compile()
    return nc


def _prep_consts(p, scale):
    """Constant matrices from the parameter dict; the input affine map
    (input_w/input_b) and the int18 scale are folded into the sensory
    broadcast matrix and bias."""
    iU = np.arange(U)
    sigB = np.zeros((U, U * U), np.float32)
    sigB[iU[:, None], iU[:, None] * U + iU[None, :]] = p["sigma"]
    negmusig = (-(p["mu"] * p["sigma"]).reshape(U * U, 1)).astype(np.float32)
    wm = p["w"] * p["sparsity_mask"]
    gw = np.zeros((U * U, 43), np.float32)
    flat = np.arange(U * U)
    jj = flat % U
    gw[flat, jj] = (wm * p["erev"]).reshape(-1)
    gw[flat, 32 + jj] = wm.reshape(-1)
    i43 = np.eye(43, dtype=np.float32)

    iS = np.arange(S)
    # arg = xq*(scale*input_w*sigma) + (input_b - mu)*sigma
    sig_eff = (p["sensory_sigma"] * p["input_w"][:, None] * scale).astype(np.float32)
    sigBs = np.zeros((S, S * U), np.float32)
    sigBs[iS[:, None], iS[:, None] * U + iU[None, :]] = sig_eff
    nms = ((p["input_b"][:, None] - p["sensory_mu"]) * p["sensory_sigma"]
           ).reshape(S * U, 1).astype(np.float32)
    swm = p["sensory_w"] * p["sensory_sparsity_mask"]
    gws = np.zeros((S * U, 43), np.float32)
    sflat = np.arange(S * U)
    uu = sflat % U
    gws[sflat, uu] = (swm * p["sensory_erev"]).reshape(-1)
    gws[sflat, 32 + uu] = swm.reshape(-1)

    aug = np.zeros((1, 43), np.float32)
    aug[0, :U] = p["gleak"] * p["vleak"]
    aug[0, 32:43] = p["gleak"] + EPS
    cm6 = (UNFOLDS * p["cm"] / scale).reshape(1, U).astype(np.float32)

    mats = {
        "sigB": sigB, "negmusig": negmusig, "gw": gw, "i43": i43,
        "sigBsA": sigBs[:, :88], "sigBsB": sigBs[:, 88:],
        "nmsA": nms[:88], "nmsB": nms[88:],
        "gwsA": gws[:88], "gwsB": gws[88:],
        "aug": aug, "cm6": cm6,
    }
    cbf = np.zeros(CB_FLAT, np.float32)
    for k, (r, _o, n, fo) in CB_LAYOUT.items():
        cbf[fo:fo + r * n] = np.ascontiguousarray(mats[k], np.float32).reshape(-1)
    return cbf


def _get_runner(nc):
    """Build (once) a cached jitted shard_map executor for nc — the
    equivalent of bass_utils.run_bass_kernel_spmd's axon path, minus the
    per-call retrace/re-jit."""
    import jax
    from jax.sharding import Mesh, PartitionSpec, NamedSharding
    from jax.experimental.shard_map import shard_map
    import concourse.mybir as mybir
    from concourse import bass2jax

    bass2jax.install_neuronx_cc_hook()

    partition_name = nc.partition_id_tensor.name if nc.partition_id_tensor else None
    in_names, out_names, out_avals, zero_outs = [], [], [], []
    for alloc in nc.m.functions[0].allocations:
        if not isinstance(alloc, mybir.MemoryLocationSet):
            continue
        name = alloc.memorylocations[0].name
        if alloc.kind == "ExternalInput":
            if name != partition_name:
                in_names.append(name)
        elif alloc.kind == "ExternalOutput":
            shape = tuple(alloc.tensor_shape)
            dtype = mybir.dt.np(alloc.dtype)
            out_names.append(name)
            out_avals.append(jax.core.ShapedArray(shape, dtype))
            zero_outs.append(np.zeros(shape, dtype))
    n_params = len(in_names)
    n_outs = len(out_avals)
    in_names_all = list(in_names) + out_names
    if partition_name is not None:
        in_names_all.append(partition_name)
    donate = tuple(range(n_params, n_params + n_outs))

    def _body(*args):
        operands = list(args)
        if partition_name is not None:
            operands.append(bass2jax.partition_id_tensor())
        outs = bass2jax._bass_exec_p.bind(
            *operands,
            out_avals=tuple(out_avals),
            in_names=tuple(in_names_all),
            out_names=tuple(out_names),
            lowering_input_output_aliases=(),
            sim_require_finite=True,
            sim_require_nnan=True,
            nc=nc,
        )
        return tuple(outs)

    devices = jax.devices()[:NCORES]
    assert len(devices) == NCORES
    mesh = Mesh(np.asarray(devices), ("core",))
    in_specs = (PartitionSpec("core"),) * (n_params + n_outs)
    out_specs = (PartitionSpec("core"),) * n_outs
    sharded = jax.jit(
        shard_map(_body, mesh=mesh, in_specs=in_specs, out_specs=out_specs,
                  check_rep=False),
        donate_argnums=donate,
        keep_unused=True,
    )
    sh = NamedSharding(mesh, PartitionSpec("core"))
    return sharded, in_names, out_names, zero_outs, devices, sh


def _schedule(total):
    """Split `total` (a positive multiple of 512) into call sizes from
    {1024, 512}, preferring 1024 pieces but ending in 512s so the tail
    call (and per-core granularity overshoot) is small."""
    sizes = []
    rest = total
    while rest > 1536:
        sizes.append(1024)
        rest -= 1024
    while rest > 0:
        sizes.append(512)
        rest -= 512
    return sizes


def _get_built(size):
    key = (size, CHUNK)
    if key not in _cache:
        nc = _build(size, CHUNK)
        _cache[key] = (nc, _get_runner(nc))
    return _cache[key]


_dummies = {}


def _dummy_piece(jax, cid, dev, size):
    """Device-resident filler for cores that never ship a piece of this
    size: features zero, elapsed-row positive (avoids inf/nan in the
    recurrence; the gather mask is always 0 for these cores)."""
    k = (cid, size)
    if k not in _dummies:
        hi = np.zeros((F, size * BC), np.int16)
        hi[S, :] = 1
        lp = np.zeros((F, size * BC // 4), np.uint8)
        _dummies[k] = (jax.device_put(hi, dev), jax.device_put(lp, dev))
    return _dummies[k]


def _encode_piece(x, bidx, start, size, inv):
    """int18-encode one (core, segment) slice: int16 hi + 2-bit lo packed
    4-per-byte (byte j of a 512-col chunk holds cols j, j+128, j+256,
    j+384)."""
    xs = x[bidx, start:start + size, :]                  # [BC, size, F]
    q = np.rint(xs * inv).astype(np.int32)
    hi = np.ascontiguousarray(
        (q >> 2).astype(np.int16).transpose(2, 1, 0)
    ).reshape(F, size * BC)
    lo2 = np.ascontiguousarray(
        (q & 0x3).astype(np.uint8).transpose(2, 1, 0)
    ).reshape(F, size * BC // W, 4, W // 4)
    lo = np.ascontiguousarray(
        lo2[:, :, 0, :] | (lo2[:, :, 1, :] << 2)
        | (lo2[:, :, 2, :] << 4) | (lo2[:, :, 3, :] << 6)
    ).reshape(F, size * BC // 4)
    return hi, lo


_dev_cache = {}


def _cached_dev(jax, key, builder, target):
    """Device-array cache for per-call-constant uploads (consts, zeros,
    ramps) keyed by content hash — saves ~0.6MB of tunnel per warm call.
    Entries are never donated or mutated, so reuse is safe."""
    if key not in _dev_cache:
        if len(_dev_cache) > 64:
            _dev_cache.clear()
        _dev_cache[key] = jax.device_put(builder(), target)
    return _dev_cache[key]


def kernel(**inputs):
    import jax
    import hashlib

    p = {}
    for k, v in inputs.items():
        a = np.asarray(v)
        p[k] = a if a.dtype in (np.int64, np.int32) else np.asarray(a, np.float32)
    seq_lengths = np.asarray(inputs["seq_lengths"]).astype(np.int64)
    x = np.asarray(inputs["inputs"], np.float32)            # [B, T, F]

    # sort batch by seq_len so short sequences share a core and late
    # segments need no data at all for short cores
    perm = np.argsort(seq_lengths, kind="stable")
    seq_s = seq_lengths[perm]
    tmax_core = seq_s.reshape(NCORES, BC).max(axis=1)
    total = int(-(-int(tmax_core.max()) // 512) * 512)
    sizes = _schedule(total)

    nc, (sharded, in_names, out_names, zero_outs, devs, shd) = _get_built(sizes[0])

    # int18 encode: x ~= (hi*4 + lo_2bit) * scale
    # (threaded abs-max: numpy reductions release the GIL)
    import concurrent.futures as _cf
    xf = x.reshape(-1)
    nth = 8
    stride = (xf.shape[0] + nth - 1) // nth
    with _cf.ThreadPoolExecutor(nth) as _ex:
        mins = list(_ex.map(lambda i: float(xf[i * stride:(i + 1) * stride].min()),
                            range(nth)))
        maxs = list(_ex.map(lambda i: float(xf[i * stride:(i + 1) * stride].max()),
                            range(nth)))
    scale = max(max(maxs), -min(mins), 1e-30) / QMAX
    inv = 1.0 / scale

    pk = hashlib.blake2b(
        b"".join(p[k].tobytes() for k in sorted(p) if k != "inputs")
        + np.float64(scale).tobytes(), digest_size=16).hexdigest()
    cbf_dev = _cached_dev(
        jax, ("cbf", pk),
        lambda: np.tile(_prep_consts(p, scale).reshape(1, CB_FLAT),
                        (NCORES, 1)), shd)
    sk_h = hashlib.blake2b(seq_s.tobytes(), digest_size=16).hexdigest()
    tsel_dev = _cached_dev(
        jax, ("tsel", sk_h),
        lambda: (seq_s - 1).reshape(NCORES, BC).astype(np.float32), shd)
    v_dev = _cached_dev(
        jax, ("v0",), lambda: np.zeros((NCORES * U, BC), np.float32), shd)
    y_dev = _cached_dev(
        jax, ("y0",), lambda: np.zeros((NCORES, BC), np.float32), shd)

    last_piece = {}   # (cid, size) -> (hi_dev, lo_dev): reusable filler
    start = 0
    for size in sizes:
        nc, (sharded, in_names, out_names, zero_outs, devs, shd) = \
            _get_built(size)
        iy = out_names.index("ysel")
        iv = out_names.index("vout")
        extra = {}
        if nc.dbg_addr is not None:
            extra[nc.dbg_addr.name] = _cached_dev(
                jax, ("dbg",),
                lambda: np.tile(np.zeros((1, 2), np.uint32), (NCORES, 1)),
                shd)

        hi_shards, lo_shards = [], []
        for cid in range(NCORES):
            if start < tmax_core[cid]:
                bidx = perm[cid * BC:(cid + 1) * BC]
                hi, lo = _encode_piece(x, bidx, start, size, inv)
                hi_dev = jax.device_put(hi, devs[cid])
                lo_dev = jax.device_put(lo, devs[cid])
                last_piece[(cid, size)] = (hi_dev, lo_dev)
            elif (cid, size) in last_piece:
                # inactive core: rerun on stale data (finite garbage; its
                # gather already completed so ysel stays untouched)
                hi_dev, lo_dev = last_piece[(cid, size)]
            else:
                hi_dev, lo_dev = _dummy_piece(jax, cid, devs[cid], size)
            hi_shards.append(hi_dev)
            lo_shards.append(lo_dev)

        hi_g = jax.make_array_from_single_device_arrays(
            (NCORES * F, size * BC), shd, hi_shards)
        lo_g = jax.make_array_from_single_device_arrays(
            (NCORES * F, size * BC // 4), shd, lo_shards)
        tg = _cached_dev(
            jax, ("tg", start, size),
            lambda: np.tile((start + np.arange(size, dtype=np.float32)
                             ).reshape(1, size), (NCORES, 1)), shd)

        feeds = {"hi": hi_g, "lp": lo_g, "cbf": cbf_dev, "tsel": tsel_dev,
                 "tg": tg, "vin": v_dev, "yin": y_dev}
        feeds.update(extra)
        args = [feeds[n] for n in in_names]
        zeros_g = [np.zeros((NCORES * z.shape[0],) + z.shape[1:], z.dtype)
                   for z in zero_outs]
        outs = sharded(*args, *zeros_g)
        v_dev = outs[iv]
        y_dev = outs[iy]
        start += size

    try:
        y_dev.copy_to_host_async()
    except Exception:
        pass
    ysel = np.asarray(y_dev).reshape(NCORES, BC)

    seq_vals = ysel.reshape(B) * p["output_w"][0] + p["output_b"][0]
    out_sorted = seq_vals * p["dense_w"][0, 0] + p["dense_b"][0]
    out = np.empty(B, np.float32)
    out[perm] = out_sorted
    return out.reshape(B, 1, 1).astype(np.float32)


# revision 44
# speedup vs baseline: 1.0789x; 1.0423x over previous
"""LTC/NCP RNN (BasicRNNClassifier) Trainium2 Bass kernel.

Strategy: pure data parallel over batch (256 -> 8 cores x 32).
Per core, the sequential T=4096 recurrence runs with:
  - synapse pairs (i,j) laid out on 121 SBUF partitions
  - PE matmuls for partition-broadcast of v (sigma folded into the
    broadcast matrix) and for the masked/weighted reductions over i
    (w*mask*(erev|1) folded into a constant [121,22] matrix)
  - ACT sigmoid with per-partition bias (-mu*sigma)
  - DVE for the semi-implicit Euler update (mul/add/reciprocal/mul)
  - sensory synapses are v-independent: batched per 16-step chunk

Host->device transfer over the axon tunnel (~40MB/s, ~80ms RTT) is the
wall-clock bottleneck, so:
  - inputs ship as int18 fixed point (int16 high + 2-bit low packed
    4/byte = 2.25B/elem). The recurrence sits on a chaotic error floor:
    int18/int20/int24 all land at ~1e-3 output error vs the 2e-2 gate,
    while int16/int17 blow up to ~1e-2 — int18 is the cheapest safe
    format. The input affine map and the quantization scale are folded
    into the sensory constants.
  - the output only depends on t < seq_len[b], so the batch is sorted by
    seq_len, grouped 32-per-core, and the time axis is cut into 1024-step
    segments; a segment is only encoded+transferred for cores whose max
    seq_len reaches it (~62% of bytes on uniform lengths). v and the
    gathered output chain between segment calls as device-resident
    arrays (no per-segment host round trips); inactive cores re-run on
    their segment-0 data (finite garbage, masked out of the gather).
  - the seq_lengths gather runs on device and only [1,BC] per core is
    fetched once at the end (each blocking fetch costs a full RTT);
    the PJRT executable is built once and cached (run_bass_kernel_spmd
    re-jits per call).
"""

import numpy as np

U = 11
S = 15
F = 16
MOTOR = 1
UNFOLDS = 6
EPS = 1e-8
B, T = 256, 4096
NCORES = 8
BC = B // NCORES          # 32 batch per core
CHUNK = 16                # timesteps per inner loop iteration
SEG = 1024                # timesteps per device call (transfer skip unit)
W = CHUNK * BC            # 512 columns per chunk
QMAX = float(2**17 - 1)   # int18 quantization (int16 hi + 2-bit lo, 4/byte)


# packed constant block: name -> (rows, col_offset, cols, flat_offset)
_sizes = [("sigB", U, U * U), ("gw", U * U, 43), ("i43", 43, 43),
          ("sigBsA", S, 88), ("sigBsB", S, 77), ("gwsA", 88, 43),
          ("gwsB", 77, 43), ("aug", 1, 43), ("cm6", 1, U),
          ("negmusig", U * U, 1), ("nmsA", 88, 1), ("nmsB", 77, 1)]
CB_LAYOUT = {}
_off = 0
_foff = 0
for _n, _r, _c in _sizes:
    CB_LAYOUT[_n] = (_r, _off, _c, _foff)
    _off += _c
    _foff += _r * _c
CB_COLS = _off
CB_FLAT = _foff

_cache = {}


def _build(t_steps, chunk):
    import concourse.bass as bass
    import concourse.tile as tile
    import concourse.mybir as mybir
    from concourse import bacc
    from contextlib import ExitStack

    import concourse.tile_sem_assignment as _tsa
    _tsa.NUM_HWDGE_SEMS = 1   # keep the loop back-edge barrier under the
                              # per-instruction sync-wait limit

    f32 = mybir.dt.float32
    i16 = mybir.dt.int16
    u8 = mybir.dt.uint8
    nch = t_steps // chunk
    w = chunk * BC

    nc = bacc.Bacc("TRN2", target_bir_lowering=False, debug=False)

    hi_d = nc.dram_tensor("hi", [F, t_steps * BC], i16, kind="ExternalInput").ap()
    lp_d = nc.dram_tensor("lp", [F, t_steps * BC // 4], u8,
                          kind="ExternalInput").ap()
    cbf_d = nc.dram_tensor("cbf", [1, CB_FLAT], f32, kind="ExternalInput").ap()
    tsel_d = nc.dram_tensor("tsel", [1, BC], f32, kind="ExternalInput").ap()
    tg_d = nc.dram_tensor("tg", [1, t_steps], f32, kind="ExternalInput").ap()
    vin_d = nc.dram_tensor("vin", [U, BC], f32, kind="ExternalInput").ap()
    yin_d = nc.dram_tensor("yin", [1, BC], f32, kind="ExternalInput").ap()
    ysel_d = nc.dram_tensor("ysel", [1, BC], f32, kind="ExternalOutput").ap()
    vout_d = nc.dram_tensor("vout", [U, BC], f32, kind="ExternalOutput").ap()

    with ExitStack() as ctx:
        tc = ctx.enter_context(tile.TileContext(nc))

        cpool = ctx.enter_context(tc.tile_pool(name="consts", bufs=1))
        vpool = ctx.enter_context(tc.tile_pool(name="vstate", bufs=1))
        xpool = ctx.enter_context(tc.tile_pool(name="xin", bufs=2))
        spool = ctx.enter_context(tc.tile_pool(name="sens", bufs=2))
        apool = ctx.enter_context(tc.tile_pool(name="acts", bufs=3))
        tpool = ctx.enter_context(tc.tile_pool(name="tmps", bufs=3))
        pp_s = ctx.enter_context(tc.tile_pool(name="ps_sens", bufs=1, space="PSUM"))
        pp_u = ctx.enter_context(tc.tile_pool(name="ps_unf", bufs=2, space="PSUM"))

        cb = cpool.tile([128, CB_COLS], f32, tag="cb")
        for _name, (r, o, n, fo) in CB_LAYOUT.items():
            nc.sync.dma_start(cb[0:r, o:o + n], cbf_d[0:1, fo:fo + r * n])
        c = {k: cb[0:r, o:o + n] for k, (r, o, n, _fo) in CB_LAYOUT.items()}

        tsel_sb = cpool.tile([1, BC], f32, tag="tsel")
        nc.sync.dma_start(tsel_sb[:], tsel_d[:])

        # 2-bit-unpack constants (per-partition uint8 scalars so the
        # bitwise ALU ops stay in the integer domain)
        cnib = cpool.tile([33, 4], u8, tag="cnib")
        nc.vector.memset(cnib[:, 0:1], 3)
        nc.vector.memset(cnib[:, 1:2], 2)
        nc.vector.memset(cnib[:, 2:3], 4)
        nc.vector.memset(cnib[:, 3:4], 6)

        ones = cpool.tile([1, w], f32, tag="ones")
        nc.vector.memset(ones[:], 1.0)
        va = vpool.tile([U, BC], f32, tag="va")
        vb = vpool.tile([U, BC], f32, tag="vb")
        nc.sync.dma_start(va[:], vin_d[:])
        ysel = vpool.tile([1, BC], f32, tag="ysel")
        nc.sync.dma_start(ysel[:], yin_d[:])

        sig = mybir.ActivationFunctionType.Sigmoid
        cp = mybir.ActivationFunctionType.Copy
        mult = mybir.AluOpType.mult
        add = mybir.AluOpType.add
        iseq = mybir.AluOpType.is_equal

        with tc.For_i(0, nch, 1,
                      hint_engines=(mybir.EngineType.PE, mybir.EngineType.DVE)) as ci:
            # engine reads must start at partition 0/32/64/96, so the
            # elapsed-time row lands on partition 32 (features on 0..14)
            hi_sb = xpool.tile([33, w], i16, tag="hi")
            nc.sync.dma_start(hi_sb[0:S, :], hi_d[0:S, bass.ts(ci, w)])
            nc.sync.dma_start(hi_sb[32:33, :], hi_d[S:S + 1, bass.ts(ci, w)])
            qw = w // 4
            lp_sb = xpool.tile([33, qw], u8, tag="lp")
            nc.sync.dma_start(lp_sb[0:S, :], lp_d[0:S, bass.ts(ci, qw)])
            nc.sync.dma_start(lp_sb[32:33, :], lp_d[S:S + 1, bass.ts(ci, qw)])
            tg_sb = xpool.tile([1, chunk], f32, tag="tg")
            nc.sync.dma_start(tg_sb[:], tg_d[:, bass.ts(ci, chunk)])

            # unpack 2-bit fields: byte j packs cols (j, j+128, j+256,
            # j+384) of this chunk
            band = mybir.AluOpType.bitwise_and
            bshr = mybir.AluOpType.logical_shift_right
            lo_sb = xpool.tile([33, w], u8, tag="lo")
            for base in (0, 32):
                pr = slice(base, base + (S if base == 0 else 1))
                nc.vector.tensor_scalar(lo_sb[pr, 0:qw], lp_sb[pr, :],
                                        cnib[pr, 0:1], None, band)
                nc.vector.tensor_scalar(lo_sb[pr, qw:2 * qw], lp_sb[pr, :],
                                        cnib[pr, 1:2], cnib[pr, 0:1],
                                        bshr, op1=band)
                nc.vector.tensor_scalar(lo_sb[pr, 2 * qw:3 * qw], lp_sb[pr, :],
                                        cnib[pr, 2:3], cnib[pr, 0:1],
                                        bshr, op1=band)
                nc.vector.tensor_scalar(lo_sb[pr, 3 * qw:w], lp_sb[pr, :],
                                        cnib[pr, 3:4], None, bshr)

            # decode int18: xq = hi*4 + lo (exact in f32; scale folded
            # into sigBs/cm6 constants)
            xq = xpool.tile([33, w], f32, tag="xq")
            nc.scalar.activation(xq[0:S, :], hi_sb[0:S, :], cp, scale=4.0)
            nc.vector.scalar_tensor_tensor(xq[0:S, :], lo_sb[0:S, :], 1.0,
                                           xq[0:S, :], op0=mult, op1=add)
            nc.scalar.activation(xq[32:33, :], hi_sb[32:33, :], cp, scale=4.0)
            nc.vector.scalar_tensor_tensor(xq[32:33, :], lo_sb[32:33, :], 1.0,
                                           xq[32:33, :], op0=mult, op1=add)

            # sensory synapses, batched over the whole chunk
            pA = pp_s.tile([88, w], f32, tag="pA")
            nc.tensor.matmul(pA[:], c["sigBsA"][:], xq[0:S, :], start=True, stop=True)
            aA = spool.tile([88, w], f32, tag="aA")
            nc.scalar.activation(aA[:], pA[:], sig, bias=c["nmsA"][:])
            pB = pp_s.tile([77, w], f32, tag="pB")
            nc.tensor.matmul(pB[:], c["sigBsB"][:], xq[0:S, :], start=True, stop=True)
            aB = spool.tile([77, w], f32, tag="aB")
            nc.scalar.activation(aB[:], pB[:], sig, bias=c["nmsB"][:])

            p_nd1 = pp_s.tile([43, w], f32, tag="pnd1")
            nc.tensor.matmul(p_nd1[:], c["gwsA"][:], aA[:], start=True, stop=False)
            nc.tensor.matmul(p_nd1[:], c["gwsB"][:], aB[:], start=False, stop=False)
            nc.tensor.matmul(p_nd1[:], c["aug"][:], ones[:], start=False, stop=True)

            # cm_t = (UNFOLDS*cm/scale) * (1/xq_elapsed)
            rec = tpool.tile([1, w], f32, tag="rec")
            nc.vector.reciprocal(rec[:], xq[32:33, :])
            p_cm = pp_s.tile([U, w], f32, tag="pcm")
            nc.tensor.matmul(p_cm[:], c["cm6"][:], rec[:], start=True, stop=True)
            cmt = spool.tile([U, w], f32, tag="cmt")
            nc.vector.tensor_copy(cmt[:], p_cm[:])

            nd1 = spool.tile([43, w], f32, tag="nd1")
            nc.vector.tensor_copy(nd1[:], p_nd1[:])
            nc.vector.tensor_add(nd1[32:43, :], p_nd1[32:43, :], cmt[:])

            vcur = va
            for s in range(chunk):
                col = slice(s * BC, (s + 1) * BC)
                for k in range(UNFOLDS):
                    p_nd = pp_u.tile([43, BC], f32, tag="pnd")
                    nc.tensor.matmul(p_nd[:], c["i43"][:], nd1[:, col],
                                     start=True, stop=False)
                    p_vr = pp_u.tile([U * U, BC], f32, tag="pvr")
                    nc.tensor.matmul(p_vr[:], c["sigB"][:], vcur[:],
                                     start=True, stop=True)
                    act = apool.tile([U * U, BC], f32, tag="act")
                    nc.scalar.activation(act[:], p_vr[:], sig, bias=c["negmusig"][:])
                    nc.tensor.matmul(p_nd[:], c["gw"][:], act[:],
                                     start=False, stop=True)

                    t1 = tpool.tile([U, BC], f32, tag="t1")
                    nc.vector.tensor_mul(t1[:], cmt[:, col], vcur[:])
                    numer = tpool.tile([U, BC], f32, tag="numer")
                    nc.vector.tensor_add(numer[:], t1[:], p_nd[0:U, :])
                    rcp = tpool.tile([U, BC], f32, tag="rcp")
                    nc.vector.reciprocal(rcp[:], p_nd[32:43, :])
                    vnext = vb if k % 2 == 0 else va
                    nc.vector.tensor_mul(vnext[:], numer[:], rcp[:])
                    vcur = vnext

                # seq_lengths gather: ysel += v_motor * (t == tsel)
                mask = tpool.tile([1, BC], f32, tag="mask")
                nc.vector.tensor_scalar(mask[:], tsel_sb[:], tg_sb[0:1, s:s + 1],
                                        None, iseq)
                sel = tpool.tile([1, BC], f32, tag="sel")
                nc.vector.tensor_mul(sel[:], mask[:], vcur[0:1, :])
                nc.vector.tensor_add(ysel[:], ysel[:], sel[:])

        nc.sync.dma_start(ysel_d[:], ysel[:])
        nc.sync.dma_start(vout_d[:], va[:])

    nc.compile()
    return nc


def _prep_consts(p, scale):
    """Constant matrices from the parameter dict; the input affine map
    (input_w/input_b) and the int18 scale are folded into the sensory
    broadcast matrix and bias."""
    iU = np.arange(U)
    sigB = np.zeros((U, U * U), np.float32)
    sigB[iU[:, None], iU[:, None] * U + iU[None, :]] = p["sigma"]
    negmusig = (-(p["mu"] * p["sigma"]).reshape(U * U, 1)).astype(np.float32)
    wm = p["w"] * p["sparsity_mask"]
    gw = np.zeros((U * U, 43), np.float32)
    flat = np.arange(U * U)
    jj = flat % U
    gw[flat, jj] = (wm * p["erev"]).reshape(-1)
    gw[flat, 32 + jj] = wm.reshape(-1)
    i43 = np.eye(43, dtype=np.float32)

    iS = np.arange(S)
    # arg = xq*(scale*input_w*sigma) + (input_b - mu)*sigma
    sig_eff = (p["sensory_sigma"] * p["input_w"][:, None] * scale).astype(np.float32)
    sigBs = np.zeros((S, S * U), np.float32)
    sigBs[iS[:, None], iS[:, None] * U + iU[None, :]] = sig_eff
    nms = ((p["input_b"][:, None] - p["sensory_mu"]) * p["sensory_sigma"]
           ).reshape(S * U, 1).astype(np.float32)
    swm = p["sensory_w"] * p["sensory_sparsity_mask"]
    gws = np.zeros((S * U, 43), np.float32)
    sflat = np.arange(S * U)
    uu = sflat % U
    gws[sflat, uu] = (swm * p["sensory_erev"]).reshape(-1)
    gws[sflat, 32 + uu] = swm.reshape(-1)

    aug = np.zeros((1, 43), np.float32)
    aug[0, :U] = p["gleak"] * p["vleak"]
    aug[0, 32:43] = p["gleak"] + EPS
    cm6 = (UNFOLDS * p["cm"] / scale).reshape(1, U).astype(np.float32)

    mats = {
        "sigB": sigB, "negmusig": negmusig, "gw": gw, "i43": i43,
        "sigBsA": sigBs[:, :88], "sigBsB": sigBs[:, 88:],
        "nmsA": nms[:88], "nmsB": nms[88:],
        "gwsA": gws[:88], "gwsB": gws[88:],
        "aug": aug, "cm6": cm6,
    }
    cbf = np.zeros(CB_FLAT, np.float32)
    for k, (r, _o, n, fo) in CB_LAYOUT.items():
        cbf[fo:fo + r * n] = np.ascontiguousarray(mats[k], np.float32).reshape(-1)
    return cbf


def _get_runner(nc):
    """Build (once) a cached jitted shard_map executor for nc — the
    equivalent of bass_utils.run_bass_kernel_spmd's axon path, minus the
    per-call retrace/re-jit."""
    import jax
    from jax.sharding import Mesh, PartitionSpec, NamedSharding
    from jax.experimental.shard_map import shard_map
    import concourse.mybir as mybir
    from concourse import bass2jax

    bass2jax.install_neuronx_cc_hook()

    partition_name = nc.partition_id_tensor.name if nc.partition_id_tensor else None
    in_names, out_names, out_avals, zero_outs = [], [], [], []
    for alloc in nc.m.functions[0].allocations:
        if not isinstance(alloc, mybir.MemoryLocationSet):
            continue
        name = alloc.memorylocations[0].name
        if alloc.kind == "ExternalInput":
            if name != partition_name:
                in_names.append(name)
        elif alloc.kind == "ExternalOutput":
            shape = tuple(alloc.tensor_shape)
            dtype = mybir.dt.np(alloc.dtype)
            out_names.append(name)
            out_avals.append(jax.core.ShapedArray(shape, dtype))
            zero_outs.append(np.zeros(shape, dtype))
    n_params = len(in_names)
    n_outs = len(out_avals)
    in_names_all = list(in_names) + out_names
    if partition_name is not None:
        in_names_all.append(partition_name)
    donate = tuple(range(n_params, n_params + n_outs))

    def _body(*args):
        operands = list(args)
        if partition_name is not None:
            operands.append(bass2jax.partition_id_tensor())
        outs = bass2jax._bass_exec_p.bind(
            *operands,
            out_avals=tuple(out_avals),
            in_names=tuple(in_names_all),
            out_names=tuple(out_names),
            lowering_input_output_aliases=(),
            sim_require_finite=True,
            sim_require_nnan=True,
            nc=nc,
        )
        return tuple(outs)

    devices = jax.devices()[:NCORES]
    assert len(devices) == NCORES
    mesh = Mesh(np.asarray(devices), ("core",))
    in_specs = (PartitionSpec("core"),) * (n_params + n_outs)
    out_specs = (PartitionSpec("core"),) * n_outs
    sharded = jax.jit(
        shard_map(_body, mesh=mesh, in_specs=in_specs, out_specs=out_specs,
                  check_rep=False),
        donate_argnums=donate,
        keep_unused=True,
    )
    sh = NamedSharding(mesh, PartitionSpec("core"))
    return sharded, in_names, out_names, zero_outs, devices, sh


def _schedule(total):
    """Split `total` (a positive multiple of 512) into call sizes from
    {1024, 512}, preferring 1024 pieces but ending in 512s so the tail
    call (and per-core granularity overshoot) is small."""
    sizes = []
    rest = total
    while rest > 1536:
        sizes.append(1024)
        rest -= 1024
    while rest > 0:
        sizes.append(512)
        rest -= 512
    return sizes


def _get_built(size):
    key = (size, CHUNK)
    if key not in _cache:
        nc = _build(size, CHUNK)
        _cache[key] = (nc, _get_runner(nc))
    return _cache[key]


_dummies = {}


def _dummy_piece(jax, cid, dev, size):
    """Device-resident filler for cores that never ship a piece of this
    size: features zero, elapsed-row positive (avoids inf/nan in the
    recurrence; the gather mask is always 0 for these cores)."""
    k = (cid, size)
    if k not in _dummies:
        hi = np.zeros((F, size * BC), np.int16)
        hi[S, :] = 1
        lp = np.zeros((F, size * BC // 4), np.uint8)
        _dummies[k] = (jax.device_put(hi, dev), jax.device_put(lp, dev))
    return _dummies[k]


def _encode_piece(x, bidx, start, size, inv):
    """int18-encode one (core, segment) slice: int16 hi + 2-bit lo packed
    4-per-byte (byte j of a 512-col chunk holds cols j, j+128, j+256,
    j+384)."""
    xs = x[bidx, start:start + size, :]                  # [BC, size, F]
    # clip guard: scale comes from a subsample, so the true max may exceed
    # range by a hair; clipped features sit in deep sigmoid saturation
    # where the quantization error vanishes
    q = np.clip(np.rint(xs * inv), -QMAX, QMAX).astype(np.int32)
    hi = np.ascontiguousarray(
        (q >> 2).astype(np.int16).transpose(2, 1, 0)
    ).reshape(F, size * BC)
    lo2 = np.ascontiguousarray(
        (q & 0x3).astype(np.uint8).transpose(2, 1, 0)
    ).reshape(F, size * BC // W, 4, W // 4)
    lo = np.ascontiguousarray(
        lo2[:, :, 0, :] | (lo2[:, :, 1, :] << 2)
        | (lo2[:, :, 2, :] << 4) | (lo2[:, :, 3, :] << 6)
    ).reshape(F, size * BC // 4)
    return hi, lo


_dev_cache = {}


def _cached_dev(jax, key, builder, target):
    """Device-array cache for per-call-constant uploads (consts, zeros,
    ramps) keyed by content hash — saves ~0.6MB of tunnel per warm call.
    Entries are never donated or mutated, so reuse is safe."""
    if key not in _dev_cache:
        if len(_dev_cache) > 64:
            _dev_cache.clear()
        _dev_cache[key] = jax.device_put(builder(), target)
    return _dev_cache[key]


def kernel(**inputs):
    import jax
    import hashlib

    p = {}
    for k, v in inputs.items():
        a = np.asarray(v)
        p[k] = a if a.dtype in (np.int64, np.int32) else np.asarray(a, np.float32)
    seq_lengths = np.asarray(inputs["seq_lengths"]).astype(np.int64)
    x = np.asarray(inputs["inputs"], np.float32)            # [B, T, F]

    # sort batch by seq_len so short sequences share a core and late
    # segments need no data at all for short cores
    perm = np.argsort(seq_lengths, kind="stable")
    seq_s = seq_lengths[perm]
    tmax_core = seq_s.reshape(NCORES, BC).max(axis=1)
    total = int(-(-int(tmax_core.max()) // 512) * 512)
    sizes = _schedule(total)

    nc, (sharded, in_names, out_names, zero_outs, devs, shd) = _get_built(sizes[0])

    # int18 encode: x ~= (hi*4 + lo_2bit) * scale
    # (threaded abs-max: numpy reductions release the GIL)
    import concurrent.futures as _cf
    xf = x.reshape(-1)
    nth = 8
    stride = (xf.shape[0] + nth - 1) // nth
    with _cf.ThreadPoolExecutor(nth) as _ex:
        mins = list(_ex.map(lambda i: float(xf[i * stride:(i + 1) * stride].min()),
                            range(nth)))
        maxs = list(_ex.map(lambda i: float(xf[i * stride:(i + 1) * stride].max()),
                            range(nth)))
    scale = max(max(maxs), -min(mins), 1e-30) / QMAX
    inv = 1.0 / scale

    pk = hashlib.blake2b(
        b"".join(p[k].tobytes() for k in sorted(p) if k != "inputs")
        + np.float64(scale).tobytes(), digest_size=16).hexdigest()
    cbf_dev = _cached_dev(
        jax, ("cbf", pk),
        lambda: np.tile(_prep_consts(p, scale).reshape(1, CB_FLAT),
                        (NCORES, 1)), shd)
    sk_h = hashlib.blake2b(seq_s.tobytes(), digest_size=16).hexdigest()
    tsel_dev = _cached_dev(
        jax, ("tsel", sk_h),
        lambda: (seq_s - 1).reshape(NCORES, BC).astype(np.float32), shd)
    v_dev = _cached_dev(
        jax, ("v0",), lambda: np.zeros((NCORES * U, BC), np.float32), shd)
    y_dev = _cached_dev(
        jax, ("y0",), lambda: np.zeros((NCORES, BC), np.float32), shd)

    last_piece = {}   # (cid, size) -> (hi_dev, lo_dev): reusable filler
    start = 0
    for size in sizes:
        nc, (sharded, in_names, out_names, zero_outs, devs, shd) = \
            _get_built(size)
        iy = out_names.index("ysel")
        iv = out_names.index("vout")
        extra = {}
        if nc.dbg_addr is not None:
            extra[nc.dbg_addr.name] = _cached_dev(
                jax, ("dbg",),
                lambda: np.tile(np.zeros((1, 2), np.uint32), (NCORES, 1)),
                shd)

        hi_shards, lo_shards = [], []
        for cid in range(NCORES):
            if start < tmax_core[cid]:
                bidx = perm[cid * BC:(cid + 1) * BC]
                hi, lo = _encode_piece(x, bidx, start, size, inv)
                hi_dev = jax.device_put(hi, devs[cid])
                lo_dev = jax.device_put(lo, devs[cid])
                last_piece[(cid, size)] = (hi_dev, lo_dev)
            elif (cid, size) in last_piece:
                # inactive core: rerun on stale data (finite garbage; its
                # gather already completed so ysel stays untouched)
                hi_dev, lo_dev = last_piece[(cid, size)]
            else:
                hi_dev, lo_dev = _dummy_piece(jax, cid, devs[cid], size)
            hi_shards.append(hi_dev)
            lo_shards.append(lo_dev)

        hi_g = jax.make_array_from_single_device_arrays(
            (NCORES * F, size * BC), shd, hi_shards)
        lo_g = jax.make_array_from_single_device_arrays(
            (NCORES * F, size * BC // 4), shd, lo_shards)
        tg = _cached_dev(
            jax, ("tg", start, size),
            lambda: np.tile((start + np.arange(size, dtype=np.float32)
                             ).reshape(1, size), (NCORES, 1)), shd)

        feeds = {"hi": hi_g, "lp": lo_g, "cbf": cbf_dev, "tsel": tsel_dev,
                 "tg": tg, "vin": v_dev, "yin": y_dev}
        feeds.update(extra)
        args = [feeds[n] for n in in_names]
        zeros_g = [np.zeros((NCORES * z.shape[0],) + z.shape[1:], z.dtype)
                   for z in zero_outs]
        outs = sharded(*args, *zeros_g)
        v_dev = outs[iv]
        y_dev = outs[iy]
        start += size

    try:
        y_dev.copy_to_host_async()
    except Exception:
        pass
    ysel = np.asarray(y_dev).reshape(NCORES, BC)

    seq_vals = ysel.reshape(B) * p["output_w"][0] + p["output_b"][0]
    out_sorted = seq_vals * p["dense_w"][0, 0] + p["dense_b"][0]
    out = np.empty(B, np.float32)
    out[perm] = out_sorted
    return out.reshape(B, 1, 1).astype(np.float32)
